# revision 37
# baseline (speedup 1.0000x reference)
"""Causal multi-head attention (B=2, T=2048, E=1024, 16 heads) on 8 TRN2 cores.

Sharding: 8-way tensor-parallel over heads (2 heads/core) for QKV projections
and attention; AllToAll re-shards the attention output over tokens so each
core computes the output projection for its 512-token block.

v3:
- All matmul operands are bf16 (PSUM accumulation stays fp32).
- Attention runs as two per-head passes. Head-0's AllToAll fires mid-kernel
  and overlaps head-1's compute; the even-head half of the output projection
  is interleaved into the head-1 pass, and head-1's AllToAll overlaps only
  the small odd-half tail.
- Softmax normalization is deferred past the AllToAll: senders ship raw
  accumulated sums plus the denominator row (from a ones column in V);
  receivers batch all 8 denominators into one reciprocal_approx_fast and
  rescale in place. This removes 16 serialized 3.4us DVE reciprocals.
- Causal masking of diagonal blocks is a second PE matmul (triT x identity)
  accumulated into the score PSUM - no vector-engine op in the
  scores->exp->AV chain.
- Scores are computed transposed (S^T = K Q^T) so softmax P^T feeds the AV
  matmul directly; scale 1/8 is folded into the exp activation; score
  sub-blocks land in [128,1024] two-bank PSUM tiles so each activation
  covers 1024 columns; max-subtraction is omitted (scores are O(1)).
"""
import sys

if "/opt/trn_rl_repo" not in sys.path:
    sys.path.insert(0, "/opt/trn_rl_repo")

import numpy as np
import ml_dtypes

import concourse.bacc as bacc
import concourse.mybir as mybir
from concourse import tile
from concourse.bass_utils import run_bass_kernel_spmd

dt = mybir.dt
AF = mybir.ActivationFunctionType
ALU = mybir.AluOpType

B, T, E, HS, NH = 2, 2048, 1024, 64, 16
NCORE = 8
NTOK = B * T            # 4096
CH = 512                # token chunk
NCH = NTOK // CH        # 8
CPB = NCH // B          # chunks per batch = 4
SUB = 128
NSUB = CH // SUB        # 4
NEG = -1.0e30
BF16 = ml_dtypes.bfloat16

_nc_cache = {}


def build_nc():
    nc = bacc.Bacc("TRN2", target_bir_lowering=False, debug=False,
                   num_devices=NCORE)
    f32, f32r, bf16 = dt.float32, dt.float32r, dt.bfloat16

    xT = nc.declare_dram_parameter("xT", [E, NTOK], bf16, isOutput=False)
    wqT = nc.declare_dram_parameter("wqT", [E, 128], bf16, isOutput=False)
    wkT = nc.declare_dram_parameter("wkT", [E, 128], bf16, isOutput=False)
    wvT = nc.declare_dram_parameter("wvT", [E, 128], bf16, isOutput=False)
    woh0 = nc.declare_dram_parameter("woh0", [512, E], bf16, isOutput=False)
    woh1 = nc.declare_dram_parameter("woh1", [512, E], bf16, isOutput=False)
    bqs = nc.declare_dram_parameter("bqs", [128, 1], f32, isOutput=False)
    bks = nc.declare_dram_parameter("bks", [128, 1], f32, isOutput=False)
    bvs = nc.declare_dram_parameter("bvs", [128, 1], f32, isOutput=False)
    bo_b = nc.declare_dram_parameter("bo_b", [128, E], f32, isOutput=False)
    eyeW = nc.declare_dram_parameter("eyeW", [128, CH], bf16, isOutput=False)
    triT = nc.declare_dram_parameter("triT", [128, 128], bf16, isOutput=False)
    sel8 = nc.declare_dram_parameter("sel8", [8, 512], f32, isOutput=False)
    y = nc.declare_dram_parameter("y", [CH, E], f32, isOutput=True)

    with tile.TileContext(nc) as tc:
        from contextlib import ExitStack
        with ExitStack() as top:
            const = top.enter_context(tc.tile_pool(name="const", bufs=1))
            persist = top.enter_context(tc.tile_pool(name="persist", bufs=1))
            xtp_pool = top.enter_context(tc.tile_pool(name="xtp", bufs=2))
            ps_q = top.enter_context(
                tc.tile_pool(name="ps_q", bufs=2, space="PSUM"))
            ps_s = top.enter_context(
                tc.tile_pool(name="ps_s", bufs=2, space="PSUM"))
            ps_a = top.enter_context(
                tc.tile_pool(name="ps_a", bufs=2, space="PSUM"))
            dram = top.enter_context(
                tc.tile_pool(name="dram", bufs=1, space="DRAM"))
            vstage = top.enter_context(tc.tile_pool(name="vstage", bufs=2))
            ppool = top.enter_context(tc.tile_pool(name="ppool", bufs=3))
            apool = top.enter_context(tc.tile_pool(name="apool", bufs=2))
            ystage = top.enter_context(tc.tile_pool(name="ystage", bufs=2))

            # ---- first chunk's inputs before anything else ----------------
            wq_sb = persist.tile([128, 8, 128], bf16, name="wq_sb")
            xT0 = xtp_pool.tile([128, 8, CH], bf16, name="xTt", tag="xTt")
            wqv = wqT.rearrange("(e p) m -> p e m", p=128)
            for e in range(8):
                nc.sync.dma_start(
                    xT0[:, e, :], xT[128 * e:128 * (e + 1), 0:CH])
                nc.sync.dma_start(wq_sb[:, e, :], wqv[:, e, :])

            # ---- constants -------------------------------------------------
            eyeW_sb = const.tile([128, CH], bf16, name="eyeW_sb")
            nc.sync.dma_start(eyeW_sb[:], eyeW[:])
            triT_sb = const.tile([128, 128], bf16, name="triT_sb")
            nc.sync.dma_start(triT_sb[:], triT[:])
            sel_sb = const.tile([8, 4, 128], f32, name="sel_sb")
            nc.sync.dma_start(sel_sb[:],
                              sel8.rearrange("k (s m) -> k s m", s=4))
            sel_r = const.tile([8, 4, 128], f32r, name="sel_r")
            nc.vector.tensor_copy(sel_r[:], sel_sb[:])
            bq_sb = const.tile([128, 1], f32, name="bq_sb")
            nc.sync.dma_start(bq_sb[:], bqs[:])
            bk_sb = const.tile([128, 1], f32, name="bk_sb")
            nc.sync.dma_start(bk_sb[:], bks[:])
            bv_sb = const.tile([128, 1], f32, name="bv_sb")
            nc.sync.dma_start(bv_sb[:], bvs[:])
            bo_sb = const.tile([128, E], f32, name="bo_sb")

            # ---- persistent tensors ---------------------------------------
            wk_sb = persist.tile([128, 8, 128], bf16, name="wk_sb")
            wv_sb = persist.tile([128, 8, 128], bf16, name="wv_sb")
            wo0_sb = persist.tile([128, 4, E], bf16, name="wo0_sb")
            wo1_sb = persist.tile([128, 4, E], bf16, name="wo1_sb")
            nc.sync.dma_start(wk_sb[:], wkT.rearrange("(e p) m -> p e m", p=128))
            nc.sync.dma_start(wv_sb[:], wvT.rearrange("(e p) m -> p e m", p=128))
            kT = persist.tile([128, NCH, CH], bf16, name="kT")
            qT = persist.tile([128, NCH, CH], bf16, name="qT")
            # per-head V rows (+ ones column for softmax denominators)
            vh0 = persist.tile([128, NCH * NSUB, 65], bf16, name="vh0")
            vh1 = persist.tile([128, NCH * NSUB, 65], bf16, name="vh1")
            nc.vector.memset(vh0[:, :, 64], 1.0)
            nc.vector.memset(vh1[:, :, 64], 1.0)

            # cc payload: 64 rows of raw attention sums + denom row, shipped
            # as ONE flat contiguous slice per peer (the gpsimd trigger's
            # descriptor processing scales with descriptor count)
            cc_in = [dram.tile([NCH, 65 * CH], bf16, name=f"cc_in{h}")
                     for h in range(2)]
            cc_out = [dram.tile([NCH, 65 * CH], bf16, name=f"cc_out{h}")
                      for h in range(2)]

            # out-projection staging (receiver side)
            aTs = [persist.tile([128, 4, CH], bf16, name=f"aT{h}")
                   for h in range(2)]
            dsb = [persist.tile([8, CH], bf16, name=f"dsb{h}")
                   for h in range(2)]
            yacc = persist.tile([128, NSUB, E], f32, name="yacc")

            # ---- phase B: QKV projection for chunk t, as 3 units ----------
            def qkv_units(t):
                xTt_box = []

                def load_x():
                    if t == 0:
                        xTt_box.append(xT0)
                        return
                    xTt = xtp_pool.tile([128, 8, CH], bf16, name="xTt",
                                        tag="xTt")
                    for e in range(8):
                        nc.sync.dma_start(
                            xTt[:, e, :],
                            xT[128 * e:128 * (e + 1), CH * t:CH * (t + 1)])
                    xTt_box.append(xTt)

                def proj(wsb, bias, dest):
                    xTt = xTt_box[0]
                    ps = ps_q.tile([128, CH], f32, name="psqk", tag="psq")
                    for e in range(8):
                        nc.tensor.matmul(ps[:], wsb[:, e, :], xTt[:, e, :],
                                         start=(e == 0), stop=(e == 7))
                    nc.vector.tensor_scalar_add(dest[:, t, :], ps[:], bias[:])

                def unit_q():
                    proj(wq_sb, bq_sb, qT)

                def unit_k():
                    proj(wk_sb, bk_sb, kT)

                def unit_v():
                    xTt = xTt_box[0]
                    psv = ps_q.tile([128, CH], f32, name="psv", tag="psq")
                    for e in range(8):
                        nc.tensor.matmul(psv[:], wv_sb[:, e, :], xTt[:, e, :],
                                         start=(e == 0), stop=(e == 7))
                    vTs = vstage.tile([128, CH], bf16, name="vTs", tag="vTs")
                    nc.vector.tensor_scalar_add(vTs[:], psv[:], bv_sb[:])
                    tva = ps_q.tile([128, CH], bf16, name="tva", tag="psq")
                    for s in range(NSUB):
                        nc.tensor.transpose(
                            tva[:, 128 * s:128 * (s + 1)],
                            vTs[:, 128 * s:128 * (s + 1)], eyeW_sb[:, 0:128])
                    tv4 = tva.rearrange("p (s x) -> p s x", s=4)
                    g0 = NSUB * t
                    nc.vector.tensor_copy(vh0[:, g0:g0 + 4, 0:64],
                                          tv4[:, :, 0:64])
                    nc.vector.tensor_copy(vh1[:, g0:g0 + 4, 0:64],
                                          tv4[:, :, 64:128])

                return [load_x, unit_q, unit_k, unit_v]

            # ---- phase C: attention for (head h, query chunk t) -----------
            # emitted as a list of units so QKV/out-proj matmuls can be
            # interleaved between the scalar-bound scores->exp->AV pairs
            def attn_units(h, t):
                b0 = CPB * (t // CPB)
                p0 = 64 * h
                vh = vh0 if h == 0 else vh1
                state = {}

                def emit_scores(kc, j):
                    diag = kc == t
                    sps = ps_s.tile([128, 2, CH], f32, name="sps", tag="sps")
                    pT = ppool.tile([128, 2, CH], bf16, name="pT", tag="pT")
                    for jj in range(2):
                        s = 2 * j + jj
                        q0 = 128 * s if diag else 0
                        nc.tensor.matmul(
                            sps[:, jj, q0:CH],
                            kT[p0:p0 + 64, kc, 128 * s:128 * (s + 1)],
                            qT[p0:p0 + 64, t, q0:CH],
                            start=True, stop=not diag)
                        if diag:
                            # accumulate the -1e30 upper-triangle into the
                            # first 128 cols via the PE (zeros elsewhere):
                            # out[k,q] += sum_j triT[j,k] * eyeW[j,q]
                            nc.tensor.matmul(
                                sps[:, jj, q0:CH],
                                triT_sb[:], eyeW_sb[:, 0:CH - q0],
                                start=False, stop=True)
                            nc.scalar.activation(pT[:, jj, q0:CH],
                                                 sps[:, jj, q0:CH],
                                                 AF.Exp, scale=0.125)
                    if not diag:
                        nc.scalar.activation(pT[:], sps[:], AF.Exp,
                                             scale=0.125)
                    return pT

                def emit_av(kc, j, pT):
                    diag = kc == t
                    a_ps = state["a_ps"]
                    for jj in range(2):
                        s = 2 * j + jj
                        q0 = 128 * s if diag else 0
                        g = NSUB * kc + s
                        nc.tensor.matmul(
                            a_ps[0:65, q0:CH], vh[:, g, :], pT[:, jj, q0:CH],
                            start=(kc == b0 and s == 0),
                            stop=(diag and s == NSUB - 1))

                def unit_pair(kc, j):
                    if kc == b0 and j == 0:
                        state["a_ps"] = ps_a.tile([128, CH], f32,
                                                  name="a_ps", tag="aps")
                    pT = emit_scores(kc, j)
                    prev = state.get("prev")
                    if prev is not None:
                        emit_av(prev[0], prev[1], prev[2])
                    state["prev"] = (kc, j, pT)

                def unit_final():
                    prev = state["prev"]
                    emit_av(prev[0], prev[1], prev[2])
                    a_sb = apool.tile([65, CH], bf16, name="a_sb", tag="asb")
                    nc.vector.tensor_copy(a_sb[:], state["a_ps"][0:65, :])
                    nc.sync.dma_start(
                        cc_in[h][t, :].rearrange("(p x) -> p x", p=65),
                        a_sb[:])

                units = []
                for kc in range(b0, t + 1):
                    for j in range(2):
                        units.append(
                            lambda kc=kc, j=j: unit_pair(kc, j))
                units.append(unit_final)
                return units

            # receiver-side batch normalize of one head-half's payload
            def emit_rx_norm(h):
                dT = apool.tile([8, CH], f32, name="dT", tag="dT")
                nc.vector.tensor_copy(dT[:], dsb[h][:])
                rT = apool.tile([8, CH], f32, name="rT", tag="rT")
                nc.vector.reciprocal_approx_fast(rT[:], dT[:])
                rTr = apool.tile([8, CH], f32r, name="rTr", tag="rTr")
                nc.vector.tensor_copy(rTr[:], rT[:])
                for p in range(4):
                    # rows 0..63 <- 1/denom[2p], rows 64..127 <- 1/denom[2p+1]
                    bc_ps = ps_q.tile([128, CH], f32, name="bc_ps", tag="psq")
                    nc.tensor.matmul(bc_ps[:], sel_r[:, p, :],
                                     rTr[:], start=True, stop=True)
                    bc_sb = apool.tile([128, CH], bf16, name="bc_sb",
                                       tag="bcs")
                    nc.vector.tensor_copy(bc_sb[:], bc_ps[:])
                    sl = aTs[h][:, p, :]
                    nc.vector.tensor_mul(sl, sl, bc_sb[:])

            def emit_oproj(h, ms=range(NSUB)):
                wo_sb = wo0_sb if h == 0 else wo1_sb
                for m in ms:
                    for nch in range(2):
                        yps = ps_q.tile([128, CH], f32, name="yps",
                                        tag="psq")
                        for p in range(4):
                            nc.tensor.matmul(
                                yps[:], aTs[h][:, p, 128 * m:128 * (m + 1)],
                                wo_sb[:, p, 512 * nch:512 * (nch + 1)],
                                start=(p == 0), stop=(p == 3))
                        if h == 0:
                            nc.vector.tensor_add(
                                yacc[:, m, 512 * nch:512 * (nch + 1)],
                                yps[:],
                                bo_sb[:, 512 * nch:512 * (nch + 1)])
                        else:
                            ysb = ystage.tile([128, CH], f32, name="ysb",
                                              tag="ysb")
                            nc.vector.tensor_add(
                                ysb[:], yps[:],
                                yacc[:, m, 512 * nch:512 * (nch + 1)])
                            nc.sync.dma_start(
                                y[128 * m:128 * (m + 1),
                                  512 * nch:512 * (nch + 1)],
                                ysb[:])

            def emit_rx_loads(h):
                ccv = cc_out[h].rearrange("s (p x) -> s p x", p=65)
                for s in range(8):
                    nc.sync.dma_start(
                        aTs[h][64 * (s % 2):64 * (s % 2) + 64, s // 2, :],
                        ccv[s, 0:64, :])
                nc.sync.dma_start(dsb[h][:], ccv[:, 64, :])

            def emit_a2a(h):
                nc.gpsimd.collective_compute(
                    "AllToAll", ALU.bypass,
                    ins=[cc_in[h].opt()], outs=[cc_out[h].opt()],
                    replica_groups=[list(range(NCORE))])
                emit_rx_loads(h)

            # ---- main schedule: merged lead-lag stream --------------------
            # per line t: QKV(t) + attn(h0, t-1) + attn(h1, t-2), with the
            # three QKV projections spread between the scalar-bound pairs so
            # the PE never drains (keeps the HAM clock warm)
            def emit_line(qs, us):
                # qs[0] (the x DMA) first for prefetch headroom, the rest
                # spread between the scalar-bound attention pairs
                if qs:
                    qs[0]()
                    qs = qs[1:]
                if not us:
                    for q in qs:
                        q()
                    return
                stride = max(1, len(us) // (len(qs) + 1)) if qs else len(us)
                qi = 0
                for i, u in enumerate(us):
                    u()
                    if qi < len(qs) and (i + 1) % stride == 0:
                        qs[qi]()
                        qi += 1
                while qi < len(qs):
                    qs[qi]()
                    qi += 1

            for t in range(NCH):
                qs = qkv_units(t)
                if t == 2:
                    for r in range(4):
                        nc.sync.dma_start(wo0_sb[:, r, :],
                                          woh0[128 * r:128 * (r + 1), :])
                        nc.sync.dma_start(wo1_sb[:, r, :],
                                          woh1[128 * r:128 * (r + 1), :])
                    nc.sync.dma_start(bo_sb[:], bo_b[:])
                us = []
                if t >= 1:
                    us += attn_units(0, t - 1)
                if t >= 3:
                    us += attn_units(1, t - 3)
                emit_line(qs, us)

            # h0 finishes; its AllToAll hides under the h1 tail chunks
            emit_line([], attn_units(0, NCH - 1))
            emit_a2a(0)
            for t in range(NCH - 3, NCH):
                emit_line([], attn_units(1, t))
            emit_a2a(1)
            # out-projection fills the PE during AllToAll#1's transfer
            emit_rx_norm(0)
            emit_oproj(0)
            emit_rx_norm(1)
            emit_oproj(1)
    nc.compile()
    return nc


def _prep_in_maps(embd_q, Wq, bq, Wk, bk, Wv, bv, Wo, bo):
    x = embd_q.reshape(NTOK, E).astype(np.float32)
    xT = np.ascontiguousarray(x.T).astype(BF16)
    eyeW = np.zeros((128, CH), dtype=BF16)
    eyeW[:, 0:128] = np.eye(128, dtype=BF16)
    r = np.arange(128)
    # triT[j, k] = NEG for j < k so (triT^T I)[k, q] = NEG for q < k
    triT = np.where(r[:, None] < r[None, :], np.float32(NEG),
                    np.float32(0.0)).astype(BF16)
    bo_b = np.ascontiguousarray(
        np.broadcast_to(bo.astype(np.float32), (128, E)))
    woTf = Wo.astype(np.float32).T  # [feat, out]
    # even/odd head layouts: aT partition q of block p holds
    # head 4p   (q < 64)  -> feats 256p + q          (even pass, +64 odd)
    # head 4p+2 (q >= 64) -> feats 256p + 128 + q-64
    idx = np.zeros((4, 128), dtype=np.int64)
    for p in range(4):
        idx[p, :64] = 256 * p + np.arange(64)
        idx[p, 64:] = 256 * p + 128 + np.arange(64)
    woh0 = np.ascontiguousarray(woTf[idx.reshape(-1)]).astype(BF16)
    woh1 = np.ascontiguousarray(woTf[(idx + 64).reshape(-1)]).astype(BF16)
    sel8 = np.zeros((8, 4, 128), dtype=np.float32)
    for p in range(4):
        sel8[2 * p, p, 0:64] = 1.0
        sel8[2 * p + 1, p, 64:128] = 1.0
    sel8 = sel8.reshape(8, 512)
    in_maps = []
    for c in range(NCORE):
        sl = slice(128 * c, 128 * (c + 1))
        in_maps.append({
            "xT": xT,
            "wqT": np.ascontiguousarray(Wq[sl].astype(np.float32).T).astype(BF16),
            "wkT": np.ascontiguousarray(Wk[sl].astype(np.float32).T).astype(BF16),
            "wvT": np.ascontiguousarray(Wv[sl].astype(np.float32).T).astype(BF16),
            "woh0": woh0,
            "woh1": woh1,
            "bqs": np.ascontiguousarray(bq[sl].reshape(128, 1),
                                        dtype=np.float32),
            "bks": np.ascontiguousarray(bk[sl].reshape(128, 1),
                                        dtype=np.float32),
            "bvs": np.ascontiguousarray(bv[sl].reshape(128, 1),
                                        dtype=np.float32),
            "bo_b": bo_b,
            "eyeW": eyeW,
            "triT": triT,
            "sel8": sel8,
        })
    return in_maps


def kernel(embd_q, Wq, bq, Wk, bk, Wv, bv, Wo, bo, _trace=False):
    if "full" not in _nc_cache:
        _nc_cache["full"] = build_nc()
    in_maps = _prep_in_maps(np.asarray(embd_q), np.asarray(Wq), np.asarray(bq),
                            np.asarray(Wk), np.asarray(bk), np.asarray(Wv),
                            np.asarray(bv), np.asarray(Wo), np.asarray(bo))
    import os
    tc_env = os.environ.get("TRACE_CORES")
    res = run_bass_kernel_spmd(
        _nc_cache["full"], in_maps, list(range(NCORE)), trace=_trace,
        trace_cores=(list(range(NCORE)) if tc_env else None))
    out = np.concatenate(
        [res.results[c]["y"] for c in range(NCORE)], axis=0)
    out = out.astype(np.float32).reshape(B, T, E)
    kernel.last_results = res
    return out


# revision 39
# speedup vs baseline: 1.2884x; 1.2884x over previous
"""Causal multi-head attention (B=2, T=2048, E=1024, 16 heads) on 8 TRN2 cores.

Sharding: 8-way tensor-parallel over heads (2 heads/core) for QKV projections
and attention; AllToAll re-shards the attention output over tokens so each
core computes the output projection for its 512-token block.

v3:
- All matmul operands are bf16 (PSUM accumulation stays fp32).
- Attention runs as two per-head passes. Head-0's AllToAll fires mid-kernel
  and overlaps head-1's compute; the even-head half of the output projection
  is interleaved into the head-1 pass, and head-1's AllToAll overlaps only
  the small odd-half tail.
- Softmax normalization is deferred past the AllToAll: senders ship raw
  accumulated sums plus the denominator row (from a ones column in V);
  receivers batch all 8 denominators into one reciprocal_approx_fast and
  rescale in place. This removes 16 serialized 3.4us DVE reciprocals.
- Causal masking of diagonal blocks is a second PE matmul (triT x identity)
  accumulated into the score PSUM - no vector-engine op in the
  scores->exp->AV chain.
- Scores are computed transposed (S^T = K Q^T) so softmax P^T feeds the AV
  matmul directly; scale 1/8 is folded into the exp activation; score
  sub-blocks land in [128,1024] two-bank PSUM tiles so each activation
  covers 1024 columns; max-subtraction is omitted (scores are O(1)).
"""
import sys

if "/opt/trn_rl_repo" not in sys.path:
    sys.path.insert(0, "/opt/trn_rl_repo")

import numpy as np
import ml_dtypes

import concourse.bacc as bacc
import concourse.mybir as mybir
from concourse import tile
from concourse.bass_utils import run_bass_kernel_spmd

dt = mybir.dt
AF = mybir.ActivationFunctionType
ALU = mybir.AluOpType

B, T, E, HS, NH = 2, 2048, 1024, 64, 16
NCORE = 8
NTOK = B * T            # 4096
CH = 512                # token chunk
NCH = NTOK // CH        # 8
CPB = NCH // B          # chunks per batch = 4
SUB = 128
NSUB = CH // SUB        # 4
NEG = -1.0e30
BF16 = ml_dtypes.bfloat16

_nc_cache = {}


def build_nc():
    nc = bacc.Bacc("TRN2", target_bir_lowering=False, debug=False,
                   num_devices=NCORE)
    f32, f32r, bf16 = dt.float32, dt.float32r, dt.bfloat16

    xT = nc.declare_dram_parameter("xT", [E, NTOK], bf16, isOutput=False)
    wqT = nc.declare_dram_parameter("wqT", [E, 128], bf16, isOutput=False)
    wkT = nc.declare_dram_parameter("wkT", [E, 128], bf16, isOutput=False)
    wvT = nc.declare_dram_parameter("wvT", [E, 128], bf16, isOutput=False)
    woh0 = nc.declare_dram_parameter("woh0", [512, E], bf16, isOutput=False)
    woh1 = nc.declare_dram_parameter("woh1", [512, E], bf16, isOutput=False)
    bqs = nc.declare_dram_parameter("bqs", [128, 1], f32, isOutput=False)
    bks = nc.declare_dram_parameter("bks", [128, 1], f32, isOutput=False)
    bvs = nc.declare_dram_parameter("bvs", [128, 1], f32, isOutput=False)
    bo_b = nc.declare_dram_parameter("bo_b", [128, E], f32, isOutput=False)
    eyeW = nc.declare_dram_parameter("eyeW", [128, CH], bf16, isOutput=False)
    triT = nc.declare_dram_parameter("triT", [128, 128], bf16, isOutput=False)
    sel8 = nc.declare_dram_parameter("sel8", [8, 512], f32, isOutput=False)
    y = nc.declare_dram_parameter("y", [CH, E], f32, isOutput=True)

    with tile.TileContext(nc) as tc:
        from contextlib import ExitStack
        with ExitStack() as top:
            const = top.enter_context(tc.tile_pool(name="const", bufs=1))
            persist = top.enter_context(tc.tile_pool(name="persist", bufs=1))
            xtp_pool = top.enter_context(tc.tile_pool(name="xtp", bufs=2))
            ps_q = top.enter_context(
                tc.tile_pool(name="ps_q", bufs=2, space="PSUM"))
            ps_s = top.enter_context(
                tc.tile_pool(name="ps_s", bufs=2, space="PSUM"))
            ps_a = top.enter_context(
                tc.tile_pool(name="ps_a", bufs=2, space="PSUM"))
            dram = top.enter_context(
                tc.tile_pool(name="dram", bufs=1, space="DRAM"))
            vstage = top.enter_context(tc.tile_pool(name="vstage", bufs=2))
            ppool = top.enter_context(tc.tile_pool(name="ppool", bufs=3))
            apool = top.enter_context(tc.tile_pool(name="apool", bufs=2))
            ystage = top.enter_context(tc.tile_pool(name="ystage", bufs=2))

            # ---- first chunk's inputs before anything else ----------------
            wq_sb = persist.tile([128, 8, 128], bf16, name="wq_sb")
            xT0 = xtp_pool.tile([128, 8, CH], bf16, name="xTt", tag="xTt")
            wqv = wqT.rearrange("(e p) m -> p e m", p=128)
            for e in range(8):
                nc.sync.dma_start(
                    xT0[:, e, :], xT[128 * e:128 * (e + 1), 0:CH])
                nc.sync.dma_start(wq_sb[:, e, :], wqv[:, e, :])

            # ---- constants -------------------------------------------------
            eyeW_sb = const.tile([128, CH], bf16, name="eyeW_sb")
            nc.sync.dma_start(eyeW_sb[:], eyeW[:])
            triT_sb = const.tile([128, 128], bf16, name="triT_sb")
            nc.sync.dma_start(triT_sb[:], triT[:])
            sel_sb = const.tile([8, 4, 128], f32, name="sel_sb")
            nc.sync.dma_start(sel_sb[:],
                              sel8.rearrange("k (s m) -> k s m", s=4))
            sel_r = const.tile([8, 4, 128], f32r, name="sel_r")
            nc.vector.tensor_copy(sel_r[:], sel_sb[:])
            bq_sb = const.tile([128, 1], f32, name="bq_sb")
            nc.sync.dma_start(bq_sb[:], bqs[:])
            bk_sb = const.tile([128, 1], f32, name="bk_sb")
            nc.sync.dma_start(bk_sb[:], bks[:])
            bv_sb = const.tile([128, 1], f32, name="bv_sb")
            nc.sync.dma_start(bv_sb[:], bvs[:])
            bo_sb = const.tile([128, E], f32, name="bo_sb")

            # ---- persistent tensors ---------------------------------------
            wk_sb = persist.tile([128, 8, 128], bf16, name="wk_sb")
            wv_sb = persist.tile([128, 8, 128], bf16, name="wv_sb")
            wo0_sb = persist.tile([128, 4, E], bf16, name="wo0_sb")
            wo1_sb = persist.tile([128, 4, E], bf16, name="wo1_sb")
            nc.sync.dma_start(wk_sb[:], wkT.rearrange("(e p) m -> p e m", p=128))
            nc.sync.dma_start(wv_sb[:], wvT.rearrange("(e p) m -> p e m", p=128))
            kT = persist.tile([128, NCH, CH], bf16, name="kT")
            qT = persist.tile([128, NCH, CH], bf16, name="qT")
            # per-head V rows (+ ones column for softmax denominators)
            vh0 = persist.tile([128, NCH * NSUB, 65], bf16, name="vh0")
            vh1 = persist.tile([128, NCH * NSUB, 65], bf16, name="vh1")
            nc.vector.memset(vh0[:, :, 64], 1.0)
            nc.vector.memset(vh1[:, :, 64], 1.0)

            # cc payload: 64 rows of raw attention sums + denom row, shipped
            # as ONE flat contiguous slice per peer (the gpsimd trigger's
            # descriptor processing scales with descriptor count)
            cc_in = [dram.tile([NCH, 65 * CH], bf16, name=f"cc_in{h}")
                     for h in range(2)]
            cc_out = [dram.tile([NCH, 65 * CH], bf16, name=f"cc_out{h}")
                      for h in range(2)]

            # out-projection staging (receiver side)
            aTs = [persist.tile([128, 4, CH], bf16, name=f"aT{h}")
                   for h in range(2)]
            dsb = [persist.tile([8, CH], bf16, name=f"dsb{h}")
                   for h in range(2)]
            yacc = persist.tile([128, NSUB, E], f32, name="yacc")

            # ---- phase B: QKV projection for chunk t, as 3 units ----------
            def qkv_units(t):
                xTt_box = []

                def load_x():
                    if t == 0:
                        xTt_box.append(xT0)
                        return
                    xTt = xtp_pool.tile([128, 8, CH], bf16, name="xTt",
                                        tag="xTt")
                    for e in range(8):
                        nc.sync.dma_start(
                            xTt[:, e, :],
                            xT[128 * e:128 * (e + 1), CH * t:CH * (t + 1)])
                    xTt_box.append(xTt)

                def proj(wsb, bias, dest):
                    xTt = xTt_box[0]
                    ps = ps_q.tile([128, CH], f32, name="psqk", tag="psq")
                    for e in range(8):
                        nc.tensor.matmul(ps[:], wsb[:, e, :], xTt[:, e, :],
                                         start=(e == 0), stop=(e == 7))
                    nc.vector.tensor_scalar_add(dest[:, t, :], ps[:], bias[:])

                def unit_q():
                    proj(wq_sb, bq_sb, qT)

                def unit_k():
                    proj(wk_sb, bk_sb, kT)

                def unit_v():
                    xTt = xTt_box[0]
                    psv = ps_q.tile([128, CH], f32, name="psv", tag="psq")
                    for e in range(8):
                        nc.tensor.matmul(psv[:], wv_sb[:, e, :], xTt[:, e, :],
                                         start=(e == 0), stop=(e == 7))
                    vTs = vstage.tile([128, CH], bf16, name="vTs", tag="vTs")
                    nc.vector.tensor_scalar_add(vTs[:], psv[:], bv_sb[:])
                    tva = ps_q.tile([128, CH], bf16, name="tva", tag="psq")
                    for s in range(NSUB):
                        nc.tensor.transpose(
                            tva[:, 128 * s:128 * (s + 1)],
                            vTs[:, 128 * s:128 * (s + 1)], eyeW_sb[:, 0:128])
                    tv4 = tva.rearrange("p (s x) -> p s x", s=4)
                    g0 = NSUB * t
                    nc.vector.tensor_copy(vh0[:, g0:g0 + 4, 0:64],
                                          tv4[:, :, 0:64])
                    nc.vector.tensor_copy(vh1[:, g0:g0 + 4, 0:64],
                                          tv4[:, :, 64:128])

                return [load_x, unit_q, unit_k, unit_v]

            # ---- phase C: attention for (head h, query chunk t) -----------
            # emitted as a list of units so QKV/out-proj matmuls can be
            # interleaved between the scalar-bound scores->exp->AV pairs
            def attn_units(h, t):
                b0 = CPB * (t // CPB)
                p0 = 64 * h
                vh = vh0 if h == 0 else vh1
                state = {}

                def emit_scores(kc, j):
                    diag = kc == t
                    sps = ps_s.tile([128, 2, CH], f32, name="sps", tag="sps")
                    pT = ppool.tile([128, 2, CH], bf16, name="pT", tag="pT")
                    for jj in range(2):
                        s = 2 * j + jj
                        q0 = 128 * s if diag else 0
                        nc.tensor.matmul(
                            sps[:, jj, q0:CH],
                            kT[p0:p0 + 64, kc, 128 * s:128 * (s + 1)],
                            qT[p0:p0 + 64, t, q0:CH],
                            start=True, stop=not diag)
                        if diag:
                            # accumulate the -1e30 upper-triangle into the
                            # first 128 cols via the PE (zeros elsewhere):
                            # out[k,q] += sum_j triT[j,k] * eyeW[j,q]
                            nc.tensor.matmul(
                                sps[:, jj, q0:CH],
                                triT_sb[:], eyeW_sb[:, 0:CH - q0],
                                start=False, stop=True)
                            nc.scalar.activation(pT[:, jj, q0:CH],
                                                 sps[:, jj, q0:CH],
                                                 AF.Exp, scale=0.125)
                    if not diag:
                        nc.scalar.activation(pT[:], sps[:], AF.Exp,
                                             scale=0.125)
                    return pT

                def emit_av(kc, j, pT):
                    diag = kc == t
                    a_ps = state["a_ps"]
                    for jj in range(2):
                        s = 2 * j + jj
                        q0 = 128 * s if diag else 0
                        g = NSUB * kc + s
                        nc.tensor.matmul(
                            a_ps[0:65, q0:CH], vh[:, g, :], pT[:, jj, q0:CH],
                            start=(kc == b0 and s == 0),
                            stop=(diag and s == NSUB - 1))

                def unit_pair(kc, j):
                    if kc == b0 and j == 0:
                        state["a_ps"] = ps_a.tile([128, CH], f32,
                                                  name="a_ps", tag="aps")
                    pT = emit_scores(kc, j)
                    prev = state.get("prev")
                    if prev is not None:
                        emit_av(prev[0], prev[1], prev[2])
                    state["prev"] = (kc, j, pT)

                def unit_final():
                    prev = state["prev"]
                    emit_av(prev[0], prev[1], prev[2])
                    a_sb = apool.tile([65, CH], bf16, name="a_sb", tag="asb")
                    nc.vector.tensor_copy(a_sb[:], state["a_ps"][0:65, :])
                    nc.sync.dma_start(
                        cc_in[h][t, :].rearrange("(p x) -> p x", p=65),
                        a_sb[:])

                units = []
                for kc in range(b0, t + 1):
                    for j in range(2):
                        units.append(
                            lambda kc=kc, j=j: unit_pair(kc, j))
                units.append(unit_final)
                return units

            # receiver-side batch normalize of one head-half's payload
            def emit_rx_norm(h):
                dT = apool.tile([8, CH], f32, name="dT", tag="dT")
                nc.vector.tensor_copy(dT[:], dsb[h][:])
                rT = apool.tile([8, CH], f32, name="rT", tag="rT")
                nc.vector.reciprocal_approx_fast(rT[:], dT[:])
                rTr = apool.tile([8, CH], f32r, name="rTr", tag="rTr")
                nc.vector.tensor_copy(rTr[:], rT[:])
                for p in range(4):
                    # rows 0..63 <- 1/denom[2p], rows 64..127 <- 1/denom[2p+1]
                    bc_ps = ps_q.tile([128, CH], f32, name="bc_ps", tag="psq")
                    nc.tensor.matmul(bc_ps[:], sel_r[:, p, :],
                                     rTr[:], start=True, stop=True)
                    bc_sb = apool.tile([128, CH], bf16, name="bc_sb",
                                       tag="bcs")
                    nc.vector.tensor_copy(bc_sb[:], bc_ps[:])
                    sl = aTs[h][:, p, :]
                    nc.vector.tensor_mul(sl, sl, bc_sb[:])

            def emit_oproj(h, ms=range(NSUB)):
                wo_sb = wo0_sb if h == 0 else wo1_sb
                for m in ms:
                    for nch in range(2):
                        yps = ps_q.tile([128, CH], f32, name="yps",
                                        tag="psq")
                        for p in range(4):
                            nc.tensor.matmul(
                                yps[:], aTs[h][:, p, 128 * m:128 * (m + 1)],
                                wo_sb[:, p, 512 * nch:512 * (nch + 1)],
                                start=(p == 0), stop=(p == 3))
                        if h == 0:
                            nc.vector.tensor_add(
                                yacc[:, m, 512 * nch:512 * (nch + 1)],
                                yps[:],
                                bo_sb[:, 512 * nch:512 * (nch + 1)])
                        else:
                            ysb = ystage.tile([128, CH], f32, name="ysb",
                                              tag="ysb")
                            nc.vector.tensor_add(
                                ysb[:], yps[:],
                                yacc[:, m, 512 * nch:512 * (nch + 1)])
                            nc.sync.dma_start(
                                y[128 * m:128 * (m + 1),
                                  512 * nch:512 * (nch + 1)],
                                ysb[:])

            def emit_rx_loads(h):
                ccv = cc_out[h].rearrange("s (p x) -> s p x", p=65)
                for s in range(8):
                    nc.sync.dma_start(
                        aTs[h][64 * (s % 2):64 * (s % 2) + 64, s // 2, :],
                        ccv[s, 0:64, :])
                nc.sync.dma_start(dsb[h][:], ccv[:, 64, :])

            def emit_a2a(h):
                nc.gpsimd.collective_compute(
                    "AllToAll", ALU.bypass,
                    ins=[cc_in[h].opt()], outs=[cc_out[h].opt()],
                    replica_groups=[list(range(NCORE))])

            # ---- main schedule: merged lead-lag stream --------------------
            # per line t: QKV(t) + attn(h0, t-1) + attn(h1, t-2), with the
            # three QKV projections spread between the scalar-bound pairs so
            # the PE never drains (keeps the HAM clock warm)
            def emit_line(qs, us):
                # qs[0] (the x DMA) first for prefetch headroom, the rest
                # spread between the scalar-bound attention pairs
                if qs:
                    qs[0]()
                    qs = qs[1:]
                if not us:
                    for q in qs:
                        q()
                    return
                stride = max(1, len(us) // (len(qs) + 1)) if qs else len(us)
                qi = 0
                for i, u in enumerate(us):
                    u()
                    if qi < len(qs) and (i + 1) % stride == 0:
                        qs[qi]()
                        qi += 1
                while qi < len(qs):
                    qs[qi]()
                    qi += 1

            for t in range(NCH):
                qs = qkv_units(t)
                if t == 2:
                    for r in range(4):
                        nc.sync.dma_start(wo0_sb[:, r, :],
                                          woh0[128 * r:128 * (r + 1), :])
                        nc.sync.dma_start(wo1_sb[:, r, :],
                                          woh1[128 * r:128 * (r + 1), :])
                    nc.sync.dma_start(bo_sb[:], bo_b[:])
                us = []
                if t >= 1:
                    us += attn_units(0, t - 1)
                if t >= 3:
                    us += attn_units(1, t - 3)
                emit_line(qs, us)

            # h0 finishes; its AllToAll hides under the h1 tail chunks
            emit_line([], attn_units(0, NCH - 1))
            emit_a2a(0)
            for t in range(NCH - 3, NCH):
                emit_line([], attn_units(1, t))
            emit_a2a(1)
            # out-projection fills the PE during AllToAll#1's transfer
            emit_rx_loads(0)
            emit_rx_norm(0)
            emit_oproj(0)
            emit_rx_loads(1)
            emit_rx_norm(1)
            emit_oproj(1)
    nc.compile()
    return nc


def _prep_in_maps(embd_q, Wq, bq, Wk, bk, Wv, bv, Wo, bo):
    x = embd_q.reshape(NTOK, E).astype(np.float32)
    xT = np.ascontiguousarray(x.T).astype(BF16)
    eyeW = np.zeros((128, CH), dtype=BF16)
    eyeW[:, 0:128] = np.eye(128, dtype=BF16)
    r = np.arange(128)
    # triT[j, k] = NEG for j < k so (triT^T I)[k, q] = NEG for q < k
    triT = np.where(r[:, None] < r[None, :], np.float32(NEG),
                    np.float32(0.0)).astype(BF16)
    bo_b = np.ascontiguousarray(
        np.broadcast_to(bo.astype(np.float32), (128, E)))
    woTf = Wo.astype(np.float32).T  # [feat, out]
    # even/odd head layouts: aT partition q of block p holds
    # head 4p   (q < 64)  -> feats 256p + q          (even pass, +64 odd)
    # head 4p+2 (q >= 64) -> feats 256p + 128 + q-64
    idx = np.zeros((4, 128), dtype=np.int64)
    for p in range(4):
        idx[p, :64] = 256 * p + np.arange(64)
        idx[p, 64:] = 256 * p + 128 + np.arange(64)
    woh0 = np.ascontiguousarray(woTf[idx.reshape(-1)]).astype(BF16)
    woh1 = np.ascontiguousarray(woTf[(idx + 64).reshape(-1)]).astype(BF16)
    sel8 = np.zeros((8, 4, 128), dtype=np.float32)
    for p in range(4):
        sel8[2 * p, p, 0:64] = 1.0
        sel8[2 * p + 1, p, 64:128] = 1.0
    sel8 = sel8.reshape(8, 512)
    in_maps = []
    for c in range(NCORE):
        sl = slice(128 * c, 128 * (c + 1))
        in_maps.append({
            "xT": xT,
            "wqT": np.ascontiguousarray(Wq[sl].astype(np.float32).T).astype(BF16),
            "wkT": np.ascontiguousarray(Wk[sl].astype(np.float32).T).astype(BF16),
            "wvT": np.ascontiguousarray(Wv[sl].astype(np.float32).T).astype(BF16),
            "woh0": woh0,
            "woh1": woh1,
            "bqs": np.ascontiguousarray(bq[sl].reshape(128, 1),
                                        dtype=np.float32),
            "bks": np.ascontiguousarray(bk[sl].reshape(128, 1),
                                        dtype=np.float32),
            "bvs": np.ascontiguousarray(bv[sl].reshape(128, 1),
                                        dtype=np.float32),
            "bo_b": bo_b,
            "eyeW": eyeW,
            "triT": triT,
            "sel8": sel8,
        })
    return in_maps


def kernel(embd_q, Wq, bq, Wk, bk, Wv, bv, Wo, bo, _trace=False):
    if "full" not in _nc_cache:
        _nc_cache["full"] = build_nc()
    in_maps = _prep_in_maps(np.asarray(embd_q), np.asarray(Wq), np.asarray(bq),
                            np.asarray(Wk), np.asarray(bk), np.asarray(Wv),
                            np.asarray(bv), np.asarray(Wo), np.asarray(bo))
    import os
    tc_env = os.environ.get("TRACE_CORES")
    res = run_bass_kernel_spmd(
        _nc_cache["full"], in_maps, list(range(NCORE)), trace=_trace,
        trace_cores=(list(range(NCORE)) if tc_env else None))
    out = np.concatenate(
        [res.results[c]["y"] for c in range(NCORE)], axis=0)
    out = out.astype(np.float32).reshape(B, T, E)
    kernel.last_results = res
    return out
